# revision 27
# baseline (speedup 1.0000x reference)
"""AntModel forward on 8 TRN2 NeuronCores (Bass/Tile, two-NEFF SPMD).

Math: the reference is three scatter-add layers with routing tables
dest_i = argmax(W_i, axis=1) and relu between layers. Counts are
non-negative, so the relus are no-ops and the routing composes:
out = x @ P1 @ P2 @ P3 = scatter of x by r = dest3[dest2[dest1]].

Distribution (8 cores, K-sharding over the 4096 source rows):

  NEFF A (memory-bound): core c streams rows [512c, 512c+512) of
  W1/W2/W3 (18 MB/core) as nine 2 MB chunks, all resident in SBUF
  (no recycling -> the two HWDGE rings stream continuously at
  ~420 GB/s; first/last chunks split into quarters so the DVE ramps
  with the stream head and drains with its tail). Per chunk the DVE
  does ONE full pass: a 64-wide block-max tensor_reduce
  ([128, 64, 64] -> [128, 64]), a tiny row-max reduce, and a
  max_index over the 64 block maxima (in_max = broadcast row max).
  The winning 256 B block is re-fetched from DRAM by an indirect
  SWDGE gather (row index = block + iota, computed on the Scalar
  engine as relu(bix + bias)) and shipped to the host, which
  resolves the within-block argmax (exact: the row max is inside
  the block, and np.argmax first-match composes with the device's
  first-match semantics). This halves DVE time vs the naive
  max+max_index double scan.
  Host: decodes tables, composes r = d3[d2[d1]], reshards r.

  NEFF B: core c builds one-hot(r) tiles via an int16 is_equal (4x DVE mode) and accumulates
  x[:, shard] @ onehot over its 512 sources on the TensorEngine
  (bf16 operands, f32 PSUM -- exact for integer counts), emitting a
  partial [256, 1024] in bf16 (partials < 256 -> exact). PSUM tiles
  drain (m,n)-eagerly so copies and output DMAs pipeline under the
  remaining matmuls. Host sums the 8 partials in f32.
"""

import numpy as np

import concourse.bacc as bacc
import concourse.tile as tile
import concourse.mybir as mybir
from concourse import bass
from concourse import bass_utils

N_CORES = 8
B = 256
S = 4096
SH = S // N_CORES  # 512 rows per core
N1, N2, N3 = 4096, 4096, 1024
P = 128
T = SH // P  # 4 groups of 128 rows per shard
F32 = mybir.dt.float32
BF16 = mybir.dt.bfloat16
U16 = mybir.dt.uint16
I16 = mybir.dt.int16
I32 = mybir.dt.int32

# gA column layout: one 64-wide gathered block per group
GA_OFF = [64 * g for g in range(12)]
GA_W = [64] * 12
GA_COLS = 768

_CACHE = {}


def _build_a():
    nc = bacc.Bacc("TRN2", target_bir_lowering=False, debug=False, num_devices=N_CORES)
    W1s = nc.dram_tensor("W1s", [SH, N1], F32, kind="ExternalInput")
    W2s = nc.dram_tensor("W2s", [SH, N2], F32, kind="ExternalInput")
    # W3 shard arrives host-row-permuted: W3p[p, r*1024 + c] = W3[r*128 + p, c]
    W3s = nc.dram_tensor("W3s", [P, 4 * N3], F32, kind="ExternalInput")
    bA = nc.dram_tensor("bA", [P, 96], U16, kind="ExternalOutput")
    gA = nc.dram_tensor("gA", [P, GA_COLS], F32, kind="ExternalOutput")

    # flat 256 B-block views for the indirect gathers
    w1v = W1s.rearrange("r (a b) -> (r a) b", b=64)  # [32768, 64]
    w2v = W2s.rearrange("r (a b) -> (r a) b", b=64)
    w3v = W3s.rearrange("r (a b) -> (r a) b", b=64)  # [8192, 64]

    with tile.TileContext(nc) as tc:
        with (
            tc.tile_pool(name="w", bufs=1) as wpool,
            tc.tile_pool(name="sm", bufs=1) as sm,
        ):
            # gather-row iota bases per group G (f32, exact small ints):
            #  G=0..3  (W1 chunk k): block-row = p*64 + 8192k + bix
            #  G=4..7  (W3 slot r):  block-row = p*64 + 16r   + bix
            #  G=8..11 (W2 chunk k): block-row = p*64 + 8192k + bix
            iota12 = sm.tile([P, 12], F32, tag="iota12")
            for (sl, pat, cm) in (((0, 4), [[8192, 4]], 64),
                                  ((4, 8), [[16, 4]], 64),
                                  ((8, 12), [[8192, 4]], 64)):
                nc.gpsimd.iota(iota12[:, sl[0] : sl[1]], pattern=pat, base=0,
                               channel_multiplier=cm,
                               allow_small_or_imprecise_dtypes=True)

            bix = sm.tile([P, 96], U16, tag="bix")

            def resolve(G, rmx_col, rcol, bm_sl, wv):
                # block(-pair) max_index -> gather row idx (ACT relu, exact)
                # -> indirect 512B/256B re-fetch -> ship to host
                nc.vector.max_index(
                    bix[:, 8 * G : 8 * G + 8],
                    rmx_col[:, rcol : rcol + 1].to_broadcast([P, 8]),
                    bm_sl,
                )
                gidx = sm.tile([P, 1], I32, tag=f"gidx{G}", name=f"gidx{G}")
                nc.scalar.activation(
                    gidx[:], bix[:, 8 * G : 8 * G + 1],
                    mybir.ActivationFunctionType.Relu,
                    bias=iota12[:, G : G + 1],
                )
                gath = sm.tile([P, GA_W[G]], F32, tag=f"gath{G}", name=f"gath{G}")
                nc.gpsimd.indirect_dma_start(
                    out=gath[:],
                    out_offset=None,
                    in_=wv[:],
                    in_offset=bass.IndirectOffsetOnAxis(ap=gidx[:, :1], axis=0),
                )
                ring[G % 2].dma_start(
                    gA[:, GA_OFF[G] : GA_OFF[G] + GA_W[G]], gath[:]
                )

            ring = [nc.sync, nc.scalar]
            ci = 0

            def load(dst, src):
                nonlocal ci
                ring[ci % 2].dma_start(dst, src)
                ci += 1

            # chunk schedule: W1 k0..3, W3, W2 k0..3; first/last chunks are
            # split into independent sub-tiles so the DVE ramps with the
            # stream head and drains with its tail
            SPLITS = {0: 4, 1: 2, 11: 4}
            tiles = {}
            for (Ws, g0) in ((W1s, 0), (W2s, 8)):
                for k in range(4):
                    G = g0 + k
                    ns = SPLITS.get(G, 1)
                    if ns > 1:
                        wq = []
                        width = 4096 // ns
                        for q in range(ns):
                            t = wpool.tile([P, width], F32, tag=f"wq{G}_{q}",
                                           name=f"wq{G}_{q}")
                            load(t[:], Ws[P * k : P * (k + 1),
                                          width * q : width * (q + 1)])
                            wq.append(t)
                        tiles[G] = wq
                    else:
                        w = wpool.tile([P, 4096], F32, tag=f"w{G}", name=f"w{G}")
                        load(w[:], Ws[P * k : P * (k + 1), :])
                        tiles[G] = w
                if g0 == 0:
                    w3 = wpool.tile([P, 4096], F32, tag="w3c", name="w3c")
                    load(w3[:], W3s[:, :])

            def scan_w12(G):
                w = tiles[G]
                Bm = sm.tile([P, 64], F32, tag=f"Bm{G}", name=f"Bm{G}")
                if isinstance(w, list):
                    ns = len(w)
                    nb = 64 // ns
                    for q in range(ns):
                        nc.vector.tensor_reduce(
                            Bm[:, nb * q : nb * (q + 1)],
                            w[q][:].rearrange("p (a b) -> p a b", b=64),
                            axis=mybir.AxisListType.X, op=mybir.AluOpType.max,
                        )
                else:
                    nc.vector.tensor_reduce(
                        Bm[:], w[:].rearrange("p (a b) -> p a b", b=64),
                        axis=mybir.AxisListType.X, op=mybir.AluOpType.max,
                    )
                rmx = sm.tile([P, 1], F32, tag=f"rmx{G}", name=f"rmx{G}")
                nc.vector.tensor_reduce(
                    rmx[:], Bm[:], axis=mybir.AxisListType.X,
                    op=mybir.AluOpType.max,
                )
                resolve(G, rmx, 0, Bm[:], w1v if G < 4 else w2v)

            for k in range(4):
                scan_w12(k)

            # W3: 4 row-slots per partition, 16 blocks each, no fold
            Bm3 = sm.tile([P, 64], F32, tag="Bm3")
            nc.vector.tensor_reduce(
                Bm3[:], w3[:].rearrange("p (a b) -> p a b", b=64),
                axis=mybir.AxisListType.X, op=mybir.AluOpType.max,
            )
            rmx3 = sm.tile([P, 4], F32, tag="rmx3")
            nc.vector.tensor_reduce(
                rmx3[:], Bm3[:].rearrange("p (r q) -> p r q", q=16),
                axis=mybir.AxisListType.X, op=mybir.AluOpType.max,
            )
            for r in range(4):
                resolve(4 + r, rmx3, r, Bm3[:, 16 * r : 16 * r + 16], w3v)

            for k in range(4):
                scan_w12(8 + k)

            nc.sync.dma_start(bA[:, :], bix[:])

    nc.compile()
    return nc


def _build_b():
    nc = bacc.Bacc("TRN2", target_bir_lowering=False, debug=False, num_devices=N_CORES)
    # x shard arrives host-permuted bf16: xTb[p, t*256 + c] = x[c, 512*core + 128t + p]
    xTb = nc.dram_tensor("xTb", [P, 4 * B], BF16, kind="ExternalInput")
    rpt = nc.dram_tensor("rpt", [P, T], F32, kind="ExternalInput")
    # out arrives permuted: out[p, m*1024 + j] = partial[m*128 + p, j]
    # (bf16 is exact: per-core partial counts stay far below 256)
    out = nc.dram_tensor("out", [P, 2 * N3], BF16, kind="ExternalOutput")

    with tile.TileContext(nc) as tc:
        with (
            tc.tile_pool(name="sm", bufs=1) as sm,
            tc.tile_pool(name="psum", bufs=1, space="PSUM") as psum,
        ):
            iot = sm.tile([P, N3], I16, tag="iot")
            nc.gpsimd.iota(iot[:], pattern=[[1, N3]], base=0, channel_multiplier=0)
            # PE warm-up while input DMAs are in flight (clock boost)
            wz = sm.tile([P, 512], BF16, tag="wz")
            nc.vector.memset(wz[:], 0.0)
            pmw = psum.tile([P, 512], F32, tag="pmw")
            for _ in range(6):
                nc.tensor.matmul(pmw[:], wz[:, 0:P], wz[:], start=True, stop=True)

            # rpt first on the sync ring: it gates the one-hots and its
            # ~2us completion receipt dominates for a 2KB transfer
            rp = sm.tile([P, T], F32, tag="rp")
            nc.sync.dma_start(rp[:], rpt[:, :])
            xall = sm.tile([P, 4 * B], BF16, tag="xall")
            nc.scalar.dma_start(xall[:], xTb[:, :])

            ohs = []
            for t in range(T):
                oh = sm.tile([P, N3], BF16, tag=f"oh{t}", name=f"oh{t}")
                nc.vector.tensor_scalar(
                    oh[:], iot[:], rp[:, t : t + 1], None, mybir.AluOpType.is_equal
                )
                ohs.append(oh)

            # (m, n)-outer so each PSUM tile finishes early and its copy +
            # output DMA pipeline under the remaining matmuls
            osb = sm.tile([P, 2 * N3], BF16, tag="osb")
            rings = [nc.sync, nc.scalar]
            for i, (m, n) in enumerate(((0, 0), (0, 1), (1, 0), (1, 1))):
                pm = psum.tile([P, 512], F32, tag=f"pm{m}{n}", name=f"pm{m}{n}")
                for t in range(T):
                    nc.tensor.matmul(
                        pm[:],
                        xall[:, 256 * t + P * m : 256 * t + P * (m + 1)],
                        ohs[t][:, 512 * n : 512 * (n + 1)],
                        start=(t == 0),
                        stop=(t == T - 1),
                    )
                sl = slice(1024 * m + 512 * n, 1024 * m + 512 * (n + 1))
                # all copies on Vector (idle after the one-hots): keeps the
                # Scalar engine ACTIVATE-free, so B skips the ~1.3us
                # ACT_TABLE_LOAD entirely
                nc.vector.tensor_copy(osb[:, sl], pm[:])
                rings[i % 2].dma_start(out[:, sl], osb[:, sl])

    nc.compile()
    return nc


def _get_kernels():
    if "a" not in _CACHE:
        _CACHE["a"] = _build_a()
        _CACHE["b"] = _build_b()
    return _CACHE["a"], _CACHE["b"]


def run_neffs(x, W1, W2, W3, trace=False, tmpdir_a=None, tmpdir_b=None):
    """Run both NEFFs; returns (out_full, exec_a_ns, exec_b_ns)."""
    import ml_dtypes

    nc_a, nc_b = _get_kernels()

    maps_a = []
    for c in range(N_CORES):
        sl = slice(SH * c, SH * (c + 1))
        w3p = (
            np.ascontiguousarray(W3[sl, :], dtype=np.float32)
            .reshape(4, P, N3).transpose(1, 0, 2).reshape(P, 4 * N3)
        )
        maps_a.append(
            {
                "W1s": np.ascontiguousarray(W1[sl, :], dtype=np.float32),
                "W2s": np.ascontiguousarray(W2[sl, :], dtype=np.float32),
                "W3s": np.ascontiguousarray(w3p),
            }
        )
    res_a = bass_utils.run_bass_kernel_spmd(
        nc_a, maps_a, core_ids=list(range(N_CORES)), trace=trace, tmpdir=tmpdir_a
    )

    # host: block idx from bA cols 8G, within-block argmax from the shipped
    # 64-value blocks, compose routes r = d3[d2[d1]]
    d1 = np.zeros(S, np.int64)
    d2 = np.zeros(S, np.int64)
    d3 = np.zeros(S, np.int64)
    for c in range(N_CORES):
        sl = slice(SH * c, SH * (c + 1))
        b = np.asarray(res_a.results[c]["bA"])[:, 0:96:8].astype(np.int64)
        gath = np.asarray(res_a.results[c]["gA"])
        idx = np.zeros((P, 12), np.int64)
        for G in range(12):
            blk = gath[:, GA_OFF[G] : GA_OFF[G] + GA_W[G]]
            w = np.argmax(blk, axis=1)
            idx[:, G] = b[:, G] * GA_W[G] + w
        d1[sl] = idx[:, 0:4].T.ravel()  # row 128k+p <- col k, partition p
        d3[sl] = idx[:, 4:8].T.ravel()
        d2[sl] = idx[:, 8:12].T.ravel()
    r_full = d3[d2[d1]]  # [4096] values in [0, 1024)

    maps_b = []
    for c in range(N_CORES):
        sl = slice(SH * c, SH * (c + 1))
        xtb = (
            np.ascontiguousarray(x[:, sl].T)
            .reshape(4, P, B).transpose(1, 0, 2).reshape(P, 4 * B)
            .astype(ml_dtypes.bfloat16)
        )
        r_c = r_full[sl].astype(np.float32)
        maps_b.append(
            {
                "xTb": xtb,
                "rpt": np.ascontiguousarray(r_c.reshape(T, P).T),
            }
        )
    res_b = bass_utils.run_bass_kernel_spmd(
        nc_b, maps_b, core_ids=list(range(N_CORES)), trace=trace, tmpdir=tmpdir_b
    )

    out = np.sum(
        [
            np.asarray(r["out"]).astype(np.float32)
            .reshape(P, 2, N3).transpose(1, 0, 2).reshape(2 * P, N3)
            for r in res_b.results
        ],
        axis=0,
        dtype=np.float64,
    ).astype(np.float32)
    exec_a = res_a.exec_time_ns
    exec_b = res_b.exec_time_ns
    return out, exec_a, exec_b


def kernel(x, W1, W2, W3):
    x = np.asarray(x)
    W1 = np.asarray(W1, dtype=np.float32)
    W2 = np.asarray(W2, dtype=np.float32)
    W3 = np.asarray(W3, dtype=np.float32)
    out, _, _ = run_neffs(x, W1, W2, W3, trace=False)
    return out
